# revision 1
# baseline (speedup 1.0000x reference)
"""Causal self-attention (B=2, T=2048, D=768, H=12) on 8 TRN2 NeuronCores.

Sharding: tensor-parallel over (batch, head) pairs. 24 pairs / 8 cores = 3
heads per core, all from one batch. Each core computes q/k in transposed
[head_dim, T] layout straight out of the QKV projection, runs causal
attention per head (scores^T = K^T.T-style matmuls, exp on ScalarE, softmax
denominator via a fused ones-column in the PV matmul), then a partial output
projection over its 3 heads' rows of w_out. The host sums the 4 partial
outputs per batch and adds b_out.
"""

import numpy as np
import ml_dtypes

import concourse.bass as bass
import concourse.bacc as bacc
import concourse.mybir as mybir
import concourse.tile as tile
from concourse.masks import make_upper_triangular
from concourse.bass_utils import run_bass_kernel_spmd

B, T, D, H, HD = 2, 2048, 768, 12, 64
NCORES = 8
HPC = 3            # heads per core
CPB = NCORES // B  # cores per batch = 4
CC = D // 128      # d_model chunks of 128 = 6
CCK = CC + 1       # contraction chunks incl. bias ones-row chunk
TW = T // 512      # token windows of 512 = 4
KC = T // 128      # k chunks of 128 = 16
SCALE = 1.0 / float(np.sqrt(HD))

BF = mybir.dt.bfloat16
F32 = mybir.dt.float32
NBF = ml_dtypes.bfloat16

EXP_BATCH = 2  # score chunks per exp call (PSUM tile = 2 banks)


def _attn_qw(nc, streams, qw, pools):
    """Emit attention for one q-window for a list of head streams.

    streams: list of dicts with keys:
      qq, kk : SBUF [128, T] tiles holding q^T/k^T (two 64-row halves)
      rows   : list of (row_base, kc_parity) "lanes"; for a 2-head pair the
               two streams each use one half; for the dup-packed single head
               both halves hold the same head so lanes alternate k-chunks.
      h      : head index (0..2) within this core
      vt     : vT3 tile
      yn     : yn tile
    """
    poolS, poolPS, poolE, poolSC, poolRB = (
        pools["S"], pools["PS"], pools["E"], pools["SC"], pools["RB"])
    mask_tri = pools["mask"]
    nchunks = 4 * (qw + 1)
    qs = qw * 512

    for st in streams:
        st["y"] = poolPS.tile([HD + 1, 512], F32, tag="ypv",
                              name=f"y_h{st['h']}_q{qw}")

    # batches of up to EXP_BATCH k-chunks, per stream. Diagonal (masked)
    # chunks go FIRST (mask chain off the window-tail critical path), in
    # ascending j so the start=True PV matmul (j=0) writes the full column
    # range before trimmed chunks accumulate sub-ranges of it.
    kc_order = list(range(4 * qw, nchunks)) + list(range(4 * qw))[::-1]
    for b0 in range(0, nchunks, EXP_BATCH):
        kcs = kc_order[b0:b0 + EXP_BATCH]
        nb = len(kcs)
        for st in streams:
            h = st["h"]
            s_ps = poolS.tile([128, EXP_BATCH, 512], F32, tag="s3",
                              name=f"s_h{h}_q{qw}_k{b0}")
            et = poolE.tile([128, EXP_BATCH, 512], BF, tag="et",
                            name=f"e_h{h}_q{qw}_k{b0}")
            # columns < 128*jmin are fully masked for every chunk in this
            # batch: skip them in both the matmuls and the exp
            jmin = min(max(0, kc - 4 * qw) for kc in kcs)
            for i in range(nb):
                kc = kcs[i]
                rb_, _ = st["rows"][kc % len(st["rows"])]
                nc.tensor.matmul(
                    s_ps[:, i, 128 * jmin:512],
                    lhsT=st["kk"][rb_:rb_ + HD, kc * 128:(kc + 1) * 128],
                    rhs=st["qq"][rb_:rb_ + HD, qs + 128 * jmin:qs + 512],
                    start=True, stop=True,
                )
            # exp over the whole batch (ScalarE), PSUM -> SBUF bf16
            nc.scalar.activation(
                out=et[:, 0:nb, 128 * jmin:512],
                in_=s_ps[:, 0:nb, 128 * jmin:512],
                func=mybir.ActivationFunctionType.Exp, scale=SCALE,
            )
            # causal masking on diagonal chunks (transition block only)
            for i in range(nb):
                kc = kcs[i]
                j = kc - 4 * qw
                if j < 0:
                    continue  # fully below diagonal: keep all
                nc.gpsimd.tensor_mul(
                    out=et[:, i, 128 * j:128 * (j + 1)],
                    in0=et[:, i, 128 * j:128 * (j + 1)],
                    in1=mask_tri,
                )
            # PV accumulation: lhsT = [V | ones] (65 cols), rhs = E^T.
            # Diagonal chunks contribute nothing below column 128*j, so the
            # rhs is trimmed; the j=0 chunk ran first with start=True and
            # wrote the full range, so sub-range accumulation is safe.
            for i in range(nb):
                kc = kcs[i]
                j = max(0, kc - 4 * qw)
                idx = b0 + i
                nc.tensor.matmul(
                    st["y"][:, 128 * j:512],
                    lhsT=st["vt"][:, kc, st["h"], 0:HD + 1],
                    rhs=et[:, i, 128 * j:512],
                    start=(idx == 0), stop=(idx == nchunks - 1),
                    skip_group_check=True,
                )

    # normalize: yn = y[0:64] / sumexp (row 64), cast to bf16.
    # st["yn_ap"](qs) gives the destination slice (may be a shifted
    # partition range -- DVE supports differing src/dst base partitions).
    F32R = mybir.dt.float32r
    for st in streams:
        h = st["h"]
        y = st["y"]
        sc = poolSC.tile([128, 512], F32R, tag="sc", name=f"sc_h{h}_q{qw}")
        rb = poolRB.tile([HD, 512], F32, tag="rb", name=f"rb_h{h}_q{qw}")
        # reciprocal of sumexp, kept on partition 64 (aligned with source),
        # written as fp32r so the broadcast matmul can consume it
        with nc.allow_low_precision(reason="fp32r == fp32 bits; rounding "
                                    "only affects the PE broadcast matmul"):
            nc.vector.reciprocal(out=sc[HD:HD + 1, :], in_=y[HD:HD + 1, :])
        # broadcast to 64 partitions with a K=1 fp32r matmul against a ones
        # row (full-rate for N>=256); then evacuate to SBUF for the multiply
        rbps = poolS.tile([HD, 512], F32, tag="s3", name=f"rbps_h{h}_q{qw}")
        nc.tensor.matmul(
            rbps,
            lhsT=pools["ones"][HD:HD + 1, 0:HD],
            rhs=sc[HD:HD + 1, :],
            start=True, stop=True,
        )
        nc.any.tensor_copy(out=rb, in_=rbps)
        nc.vector.tensor_mul(
            out=st["yn_ap"](qs),
            in0=y[0:HD, :], in1=rb[:, :],
        )


def build_bass():
    nc = bacc.Bacc(None, target_bir_lowering=False)

    xT = nc.dram_tensor("xT", [CC, 128, T], BF, kind="ExternalInput")
    wqk = nc.dram_tensor("wqk", [CCK, 128, 3, 128], BF, kind="ExternalInput")
    wv = nc.dram_tensor("wv", [CC, 128, HPC * HD], BF, kind="ExternalInput")
    wo = nc.dram_tensor("wo", [HPC, HD, D], BF, kind="ExternalInput")
    outT = nc.dram_tensor("outT", [D, T], F32, kind="ExternalOutput")

    with tile.TileContext(nc) as tc:
        with (
            tc.tile_pool(name="big", bufs=1) as big,
            tc.tile_pool(name="ets", bufs=6) as ets,
            tc.tile_pool(name="scr", bufs=4) as scr,
            tc.tile_pool(name="outs", bufs=6) as outs,
            tc.tile_pool(name="psS", bufs=2, space="PSUM") as poolS,
            tc.tile_pool(name="psA", bufs=2, space="PSUM") as poolPS,
        ):
            # ---- constants / inputs in SBUF ----
            # weights first (small), then x in token-window-major order so
            # the first token window's projection completes after ~1MB of
            # traffic instead of the full 3.7MB
            wqks = big.tile([128, CCK, 3, 128], BF, tag="wqk")
            wvs = big.tile([128, CC, HPC * HD], BF, tag="wv")
            xTs = big.tile([128, CCK, T], BF, tag="xT")
            # bias chunk: only partition 0 (the ones row) is ever read --
            # the bias matmul below uses K=1 -- so no DMA and no zero-fill
            nc.gpsimd.memset(xTs[0:1, CC, :], 1.0)
            nc.sync.dma_start(
                out=wqks[:, 0:4], in_=wqk[0:4].rearrange("c p a f -> p c a f"))
            for cc in range(4):
                nc.sync.dma_start(out=xTs[:, cc, 0:512], in_=xT[cc, :, 0:512])
            nc.sync.dma_start(
                out=wqks[:, 4:CCK],
                in_=wqk[4:CCK].rearrange("c p a f -> p c a f"))
            for cc in range(4, CC):
                nc.sync.dma_start(out=xTs[:, cc, 0:512], in_=xT[cc, :, 0:512])
            nc.sync.dma_start(out=wvs, in_=wv.rearrange("c p f -> p c f"))
            for tw in range(1, TW):
                for cc in range(CC):
                    nc.sync.dma_start(
                        out=xTs[:, cc, tw * 512:(tw + 1) * 512],
                        in_=xT[cc, :, tw * 512:(tw + 1) * 512])
            # w_out rows: heads 0+1 stacked to 128 partitions, head 2 alone
            wos01 = big.tile([128, D], BF, tag="wo01")
            nc.sync.dma_start(out=wos01,
                              in_=wo[0:2].rearrange("h p e -> (h p) e"))
            wos2 = big.tile([HD, D], BF, tag="wo2")
            nc.sync.dma_start(out=wos2, in_=wo[2])

            mask_tri = big.tile([128, 128], BF, tag="mask")
            make_upper_triangular(nc, mask_tri, val=1.0, diag=True)
            ones_stage = big.tile([128, HD], F32, tag="ones_stage")
            nc.vector.memset(ones_stage, 1.0)
            ones_t = big.tile([128, HD], mybir.dt.float32r, tag="ones")
            with nc.allow_low_precision(reason="fp32r ones for normalizer "
                                        "broadcast matmul"):
                nc.vector.tensor_copy(out=ones_t, in_=ones_stage)

            # q^T/k^T feature-chunk tiles: QQ=[h0q|h1q], KK=[h0k|h1k],
            # QQ2=[h2q|h2q], KK2=[h2k|h2k]
            qk_tiles = []
            for nm in ("QQ", "KK", "QQ2", "KK2"):
                t_ = big.tile([128, T], BF, tag=nm, name=nm)
                qk_tiles.append(t_)

            # token-major V (+ ones column), per head: [128, kc, h, 66]
            vT3 = big.tile([128, KC, HPC, 66], BF, tag="vT3")
            for h in range(HPC):
                nc.gpsimd.memset(vT3[:, :, h, HD:HD + 1], 1.0)

            # normalized attention outputs: heads 0+1 stacked on 128
            # partitions, head 2 on its own 64-partition tile
            ynA = big.tile([128, T], BF, tag="ynA")
            ynB = big.tile([HD, T], BF, tag="ynB")

            QQ, KK, QQ2, KK2 = qk_tiles
            pools = {"S": poolS, "PS": poolPS, "E": ets, "SC": scr,
                     "RB": scr, "mask": mask_tri,
                     "ones": ones_t}

            # ---- interleaved: per token-window, project then attend ----
            for tw in range(TW):
                ts_ = tw * 512
                # q^T/k^T projection for this token window.
                # fc0=[h0q|h1q], fc1=[h0k|h1k], fc2=[h2q|h2k]; fc2's halves
                # are fanned out (duplicated) into QQ2/KK2 via DVE copies so
                # h2 scores can row-pack two k-chunks.
                for fc in range(3):
                    ps = poolPS.tile([128, 512], F32, tag="acc",
                                     name=f"ps_f{fc}_t{tw}")
                    for cc in range(CC):
                        nc.tensor.matmul(
                            ps,
                            lhsT=wqks[:, cc, fc, :],
                            rhs=xTs[:, cc, ts_:ts_ + 512],
                            start=(cc == 0), stop=False,
                        )
                    nc.tensor.matmul(
                        ps,
                        lhsT=wqks[0:1, CC, fc, :],
                        rhs=xTs[0:1, CC, ts_:ts_ + 512],
                        start=False, stop=True,
                    )
                    if fc < 2:
                        nc.any.tensor_copy(
                            out=qk_tiles[fc][:, ts_:ts_ + 512], in_=ps,
                        )
                    else:
                        for dst_half in (0, HD):
                            nc.any.tensor_copy(
                                out=QQ2[dst_half:dst_half + HD, ts_:ts_ + 512],
                                in_=ps[0:HD, :],
                            )
                            nc.any.tensor_copy(
                                out=KK2[dst_half:dst_half + HD, ts_:ts_ + 512],
                                in_=ps[HD:128, :],
                            )
                # token-major V projection for this window's 4 k-chunks
                for tc_i in range(4 * tw, 4 * tw + 4):
                    psv = poolPS.tile([128, 512], F32, tag="acc",
                                      name=f"psv_{tc_i}")
                    for cc in range(CC):
                        nc.tensor.matmul(
                            psv[:, 0:HPC * HD],
                            lhsT=xTs[:, cc, tc_i * 128:(tc_i + 1) * 128],
                            rhs=wvs[:, cc, :],
                            start=(cc == 0), stop=(cc == CC - 1),
                        )
                    nc.any.tensor_copy(
                        out=vT3[:, tc_i, :, 0:HD],
                        in_=psv[:, 0:HPC * HD].rearrange(
                            "p (h d) -> p h d", h=HPC),
                    )

                # attention for q-window tw (all needed k-chunks are ready)
                qw = tw
                qs = qw * 512
                pair = [
                    {"qq": QQ, "kk": KK, "rows": [(0, 0)], "h": 0, "vt": vT3,
                     "yn_ap": lambda q: ynA[0:HD, q:q + 512]},
                    {"qq": QQ, "kk": KK, "rows": [(HD, 0)], "h": 1, "vt": vT3,
                     "yn_ap": lambda q: ynA[HD:128, q:q + 512]},
                ]
                _attn_qw(nc, pair, qw, pools)
                solo = [
                    {"qq": QQ2, "kk": KK2, "rows": [(0, 0), (HD, 0)], "h": 2,
                     "vt": vT3, "yn_ap": lambda q: ynB[0:HD, q:q + 512]},
                ]
                _attn_qw(nc, solo, qw, pools)
                # previous window's output projection sits here in the PE
                # stream: its gate (that window's normalize chain) is long
                # done, and it fills PE while this window's solo normalize
                # chain drains
                if qw >= 1:
                    _outproj(nc, qw - 1, wos01, wos2, ynA, ynB, poolPS, outs,
                             outT)

            _outproj(nc, TW - 1, wos01, wos2, ynA, ynB, poolPS, outs, outT)
    return nc


def _outproj(nc, qw, wos01, wos2, ynA, ynB, poolPS, outs, outT):
    qs = qw * 512
    for ec in range(CC):
        ops = poolPS.tile([128, 512], F32, tag="ypv",
                          name=f"ops_e{ec}_q{qw}")
        nc.tensor.matmul(
            ops,
            lhsT=wos01[:, ec * 128:(ec + 1) * 128],
            rhs=ynA[:, qs:qs + 512],
            start=True, stop=False,
        )
        nc.tensor.matmul(
            ops,
            lhsT=wos2[:, ec * 128:(ec + 1) * 128],
            rhs=ynB[:, qs:qs + 512],
            start=False, stop=True,
        )
        osb = outs.tile([128, 512], F32, tag="osb", name=f"osb_e{ec}_q{qw}")
        nc.any.tensor_copy(out=osb, in_=ops)
        nc.sync.dma_start(
            out=outT[ec * 128:(ec + 1) * 128, qs:qs + 512],
            in_=osb,
        )


def _prep_core_inputs(c, x, w_qkv, b_qkv, w_out):
    b = c // CPB
    g = c % CPB
    hs = [HPC * g + i for i in range(HPC)]

    qc = [np.arange(h * HD, (h + 1) * HD) for h in hs]
    kc_ = [D + h * HD + np.arange(HD) for h in hs]
    vc = [2 * D + h * HD + np.arange(HD) for h in hs]

    cols = np.concatenate([qc[0], qc[1], kc_[0], kc_[1], qc[2], kc_[2]])
    vcols = np.concatenate(vc)

    xT = np.ascontiguousarray(x[b].T).astype(np.float32)
    # bias row for the K=1 bias matmul lives in wqk chunk CC, row 0
    wqk = np.zeros((CCK * 128, 384), dtype=np.float32)
    wqk[0:D] = w_qkv[:, cols]
    wqk[D] = b_qkv[cols]
    wv = w_qkv[:, vcols].astype(np.float32)
    wo = np.stack([w_out[h * HD:(h + 1) * HD, :] for h in hs]).astype(NBF)

    return {
        "xT": np.ascontiguousarray(xT.astype(NBF).reshape(CC, 128, T)),
        "wqk": np.ascontiguousarray(wqk.astype(NBF).reshape(CCK, 128, 3, 128)),
        "wv": np.ascontiguousarray(wv.astype(NBF).reshape(CC, 128, HPC * HD)),
        "wo": np.ascontiguousarray(wo),
    }


_NC_CACHE = {}


def get_nc():
    if "nc" not in _NC_CACHE:
        nc = build_bass()
        nc.finalize()  # Bacc: run reg-alloc + sync-wait splitting passes
        _NC_CACHE["nc"] = nc
    return _NC_CACHE["nc"]


def kernel(x, w_qkv, b_qkv, w_out, b_out, _run_kwargs=None):
    x = np.asarray(x, dtype=np.float32)
    w_qkv = np.asarray(w_qkv, dtype=np.float32)
    b_qkv = np.asarray(b_qkv, dtype=np.float32)
    w_out = np.asarray(w_out, dtype=np.float32)
    b_out = np.asarray(b_out, dtype=np.float32)

    nc = get_nc()
    in_maps = [_prep_core_inputs(c, x, w_qkv, b_qkv, w_out)
               for c in range(NCORES)]
    kwargs = dict(_run_kwargs or {})
    res = run_bass_kernel_spmd(nc, in_maps, core_ids=list(range(NCORES)),
                               **kwargs)
    if kwargs:
        _NC_CACHE["last_results"] = res

    bv_corr = b_qkv[2 * D:3 * D] @ w_out  # [D]
    out = np.zeros((B, T, D), dtype=np.float32)
    for b in range(B):
        acc = np.zeros((T, D), dtype=np.float32)
        for g in range(CPB):
            acc += np.asarray(res.results[b * CPB + g]["outT"]).T
        out[b] = acc + (b_out + bv_corr)[None, :]
    return out


if __name__ == "__main__":
    # smoke build
    nc = build_bass()
    print("built OK; instructions:",
          sum(1 for _ in nc.m.functions[0].instructions)
          if hasattr(nc.m.functions[0], "instructions") else "?")

